# revision 9
# baseline (speedup 1.0000x reference)
"""Trainium2 Bass kernel for nn_BDFM_46428596469849.

Per-batch math (B=8, C=256, H=W=128, HW=16384):
    m   = relu(m); z = (m > 0.3)
    er  = minpool4x4(z, SAME, border=1); di = maxpool4x4(z, SAME, border=0)
    fbu = [er, 1-di, di-er]                          # [3, HW]
    mid = fbu @ F^T                                  # [3, C]
    cf  = bn_f(Wf @ F);  mid1 = mid @ cf;  mid2 = mid^T @ mid1
    out = bn_o(W_out @ [F; mid2])

The chain collapses algebraically: with sf/bf (resp. so/bo) the BN scale/bias,
    g    = mid @ (diag(sf) Wf)            # [3, C]
    u    = mid @ bf                       # [3]
    A    = mid^T @ g                      # [C, C]
    v    = mid^T @ u                      # [C]
    Weff = W1 + W2 @ A                    # [C, C]   (W_out = [W1 | W2])
    out  = diag(so) @ Weff @ F + (so*(W2@v) + bo) 1^T
so each batch element needs only: the mid reduction (one pass over F with PE
transposes), tiny C x C algebra, and one C x C x HW matmul streamed over F.

Sharding: data-parallel, one batch element per NeuronCore (8 cores).
"""

import os
import sys

for _p in ("/opt/trn_rl_repo", "/root/.axon_site/_ro/trn_rl_repo"):
    if os.path.isdir(_p) and _p not in sys.path:
        sys.path.insert(0, _p)

import numpy as np

import concourse.bass as bass
import concourse.mybir as mybir
import concourse.tile as tile
from concourse.bass_utils import run_bass_kernel_spmd
from concourse.masks import make_identity

dt = mybir.dt
AF = mybir.ActivationFunctionType
OP = mybir.AluOpType

B, C, H, W = 8, 256, 128, 128
HW = H * W
NCORES = 8
EPS = 1e-5
F32R = dt.float32r


def _split_drain_waits(nc, max_waits=1):
    # Walrus codegen rejects instructions carrying more than a couple of
    # semaphore waits (CTRL drains and DMA descriptors in particular). Hoist
    # excess waits onto preceding NoOps on the same engine queue — the queue
    # executes in order, so the waits are satisfied before the instruction.
    for f in nc.m.functions:
        for bb in f.blocks:
            new_insts = []
            for inst in bb.instructions:
                si = inst.sync_info
                if si is not None and si.on_wait and len(si.on_wait) > max_waits:
                    waits = list(si.on_wait)
                    while len(waits) > max_waits:
                        chunk, waits = waits[:max_waits], waits[max_waits:]
                        pre = mybir.InstNoOp(
                            name=f"I-wsplit-{nc.next_id()}",
                            engine=inst.engine,
                            sync_info=mybir.SyncInfo(on_wait=chunk, on_update=[]),
                        )
                        nc.inst_map[pre.name] = pre
                        new_insts.append(pre)
                    inst.sync_info = mybir.SyncInfo(
                        on_wait=waits, on_update=list(si.on_update)
                    )
                new_insts.append(inst)
            bb.instructions[:] = new_insts


def build_nc():
    from contextlib import ExitStack

    nc = bass.Bass("TRN2", target_bir_lowering=False)

    feat = nc.declare_dram_parameter("feature", [C, HW], dt.float32, isOutput=False)
    m_in = nc.declare_dram_parameter("m", [H, W], dt.float32, isOutput=False)
    wfeat = nc.declare_dram_parameter("w_feat", [C, C], dt.float32, isOutput=False)
    wout = nc.declare_dram_parameter("w_out", [C, 2 * C], dt.float32, isOutput=False)
    bnp = {}
    for pre in ("f", "o"):
        for nm in ("gamma", "beta", "mean", "var"):
            key = f"bn_{pre}_{nm}"
            bnp[key] = nc.declare_dram_parameter(key, [C], dt.float32, isOutput=False)
    out_d = nc.declare_dram_parameter("out", [C, HW], dt.float32, isOutput=True)

    with tile.TileContext(nc) as tc, ExitStack() as ctx:
        const = ctx.enter_context(tc.tile_pool(name="const", bufs=1))
        ident = const.tile([128, 128], dt.float32, name="ident")
        make_identity(nc, ident)
        ident_r = const.tile([128, 128], F32R, name="ident_r")
        nc.vector.tensor_copy(ident_r, ident)
        eps_t = const.tile([128, 1], dt.float32, name="eps_t")
        nc.vector.memset(eps_t, EPS)

        # ---- resident feature [2][128, HW] in float32r ----
        # DMA brings raw fp32 pieces into a rotating staging pool; an engine
        # copy rounds them into the resident f32r tiles (walrus requires f32r
        # matmul operands to be produced pre-rounded by an engine).
        fpool = ctx.enter_context(tc.tile_pool(name="fpool", bufs=1))
        F = [
            fpool.tile([128, HW], F32R, name=f"F{cc}", tag=f"F{cc}")
            for cc in range(2)
        ]
        NPIECE = 16
        PIECE = HW // NPIECE
        with tc.tile_pool(name="fraw_pool", bufs=4) as fraw_pool:
            for i in range(NPIECE):
                for cc in range(2):
                    fr = fraw_pool.tile([128, PIECE], dt.float32, name="fr")
                    nc.sync.dma_start(
                        out=fr,
                        in_=feat[cc * 128 : (cc + 1) * 128, i * PIECE : (i + 1) * PIECE],
                    )
                    dst = F[cc][:, i * PIECE : (i + 1) * PIECE]
                    if cc == 0:
                        nc.vector.tensor_copy(dst, fr)
                    else:
                        nc.scalar.copy(dst, fr)

        # ---- small inputs ----
        wf = []
        wo = []
        for oc in range(2):
            t = const.tile([128, C], dt.float32, name=f"wf{oc}", tag=f"wf{oc}")
            nc.sync.dma_start(out=t, in_=wfeat[oc * 128 : (oc + 1) * 128, :])
            wf.append(t)
            t2 = const.tile([128, 2 * C], dt.float32, name=f"wo{oc}", tag=f"wo{oc}")
            nc.sync.dma_start(out=t2, in_=wout[oc * 128 : (oc + 1) * 128, :])
            wo.append(t2)
        m_sb = const.tile([128, 128], dt.float32, name="m_sb")
        nc.sync.dma_start(out=m_sb, in_=m_in[:, :])

        bnt = {}
        for key, hdl in bnp.items():
            t = const.tile([128, 2], dt.float32, name=f"t_{key}", tag=f"t_{key}")
            nc.sync.dma_start(out=t, in_=hdl[:].rearrange("(t p) -> p t", p=128))
            bnt[key] = t

        # ---- BN scale/bias: s = gamma*rsqrt(var+eps), b = beta - mean*s ----
        setup = ctx.enter_context(tc.tile_pool(name="setup", bufs=1))

        def bn_prep(pre):
            s = setup.tile([128, 2], dt.float32, name=f"s_{pre}", tag=f"s_{pre}")
            b = setup.tile([128, 2], dt.float32, name=f"b_{pre}", tag=f"b_{pre}")
            tmp = setup.tile([128, 2], dt.float32, name=f"tmp_{pre}", tag=f"tmp_{pre}")
            nc.scalar.activation(
                out=tmp, in_=bnt[f"bn_{pre}_var"], func=AF.Sqrt, bias=eps_t, scale=1.0
            )
            nc.vector.reciprocal(out=tmp, in_=tmp)
            nc.vector.tensor_mul(s, bnt[f"bn_{pre}_gamma"], tmp)
            nc.vector.tensor_mul(tmp, bnt[f"bn_{pre}_mean"], s)
            nc.vector.tensor_sub(b, bnt[f"bn_{pre}_beta"], tmp)
            return s, b

        sf, bf = bn_prep("f")
        so, bo = bn_prep("o")

        # ---- morphology: separable 4x4 window (offsets -1..+2), both passes
        # along the free dim with a PE transpose in between; border = the
        # reduction identity (matches reduce_window SAME + init value) ----
        mor = ctx.enter_context(tc.tile_pool(name="mor", bufs=1))

        def pool1d_free(src, op, border, label):
            padd = mor.tile([128, 131], dt.float32, name=f"pad_{label}", tag=f"pad_{label}")
            nc.vector.memset(padd, border)
            nc.vector.tensor_copy(padd[:, 1:129], src)
            a = mor.tile([128, 130], dt.float32, name=f"a_{label}", tag=f"a_{label}")
            nc.vector.tensor_tensor(a, padd[:, 0:130], padd[:, 1:131], op)
            r = mor.tile([128, 128], dt.float32, name=f"r_{label}", tag=f"r_{label}")
            nc.vector.tensor_tensor(r, a[:, 0:128], a[:, 2:130], op)
            return r

        z = mor.tile([128, 128], dt.float32, name="z")
        nc.vector.tensor_scalar(out=z, in0=m_sb, scalar1=0.3, scalar2=None, op0=OP.is_gt)
        erw = pool1d_free(z, OP.min, 1.0, "er1")  # [h, w] pooled over w
        diw = pool1d_free(z, OP.max, 0.0, "di1")
        with tc.tile_pool(name="mor_ps", bufs=1, space="PSUM") as mor_ps:
            er_ps = mor_ps.tile([128, 128], dt.float32, name="er_ps", tag="er_ps")
            nc.tensor.transpose(er_ps, erw, ident)
            erwT = mor.tile([128, 128], dt.float32, name="erwT")
            nc.vector.tensor_copy(erwT, er_ps)
            di_ps = mor_ps.tile([128, 128], dt.float32, name="di_ps", tag="di_ps")
            nc.tensor.transpose(di_ps, diw, ident)
            diwT = mor.tile([128, 128], dt.float32, name="diwT")
            nc.vector.tensor_copy(diwT, di_ps)
        erT = pool1d_free(erwT, OP.min, 1.0, "er2")  # [w, h] pooled over h
        diT = pool1d_free(diwT, OP.max, 0.0, "di2")

        fbuT = mor.tile([128, 128, 3], F32R, name="fbuT")  # [w, h, k]
        nc.vector.tensor_copy(fbuT[:, :, 0], erT)
        nc.vector.tensor_scalar(
            out=fbuT[:, :, 1], in0=diT, scalar1=-1.0, scalar2=1.0, op0=OP.mult, op1=OP.add
        )
        nc.vector.tensor_tensor(fbuT[:, :, 2], diT, erT, OP.subtract)

        # ---- mid = fbu @ F^T via per-h PE transposes, accumulated in PSUM ----
        alg = ctx.enter_context(tc.tile_pool(name="alg", bufs=1))
        mid_sb = alg.tile([3, C], dt.float32, name="mid_sb")
        with tc.tile_pool(name="midps", bufs=1, space="PSUM") as midps:
            mid_ps = midps.tile([3, C], dt.float32, name="mid_ps")
            with tc.tile_pool(name="tr_ps", bufs=3, space="PSUM") as tr_ps_pool, \
                 tc.tile_pool(name="f1T_pool", bufs=3) as f1T_pool:
                for hp in range(64):
                    tps = tr_ps_pool.tile([128, 512], F32R, name="tps")
                    for q in range(4):
                        h = 2 * hp + q // 2
                        cc = q % 2
                        nc.tensor.transpose(
                            tps[:, q * 128 : (q + 1) * 128],
                            F[cc][:, h * 128 : (h + 1) * 128],
                            ident_r,
                        )
                    f1T = f1T_pool.tile([128, 512], F32R, name="f1T")
                    nc.scalar.copy(f1T[:, 0:256], tps[:, 0:256])
                    nc.vector.tensor_copy(f1T[:, 256:512], tps[:, 256:512])
                    for q2 in range(2):
                        h = 2 * hp + q2
                        nc.tensor.matmul(
                            mid_ps[:, :],
                            lhsT=fbuT[:, h, :],
                            rhs=f1T[:, q2 * 256 : (q2 + 1) * 256],
                            start=(h == 0),
                            stop=(h == 127),
                        )
            nc.vector.tensor_copy(mid_sb, mid_ps)

        # ---- tiny algebra: g_ext, A_ext, W2T, WeffT, beff (plain fp32) ----
        with tc.tile_pool(name="alg_ps", bufs=1, space="PSUM") as alg_ps:
            midT_sb = alg.tile([128, 6], dt.float32, name="midT_sb")
            for cc in range(2):
                mT2 = alg_ps.tile([128, 3], dt.float32, name="mT2", tag="mT2")
                nc.tensor.transpose(
                    mT2, mid_sb[:, cc * 128 : (cc + 1) * 128], ident[0:3, 0:3]
                )
                nc.vector.tensor_copy(midT_sb[:, cc * 3 : (cc + 1) * 3], mT2)

            # rhs = [diag(sf) Wf | bf] per o-chunk; g_ext = mid @ rhs  -> [3, 257]
            rhs_g = []
            for cc in range(2):
                r = alg.tile([128, C + 1], dt.float32, name=f"rhs_g{cc}", tag=f"rhs_g{cc}")
                nc.vector.tensor_scalar(
                    out=r[:, 0:C], in0=wf[cc], scalar1=sf[:, cc : cc + 1],
                    scalar2=None, op0=OP.mult,
                )
                nc.vector.tensor_copy(r[:, C : C + 1], bf[:, cc : cc + 1])
                rhs_g.append(r)
            gext_ps = alg_ps.tile([3, C + 1], dt.float32, name="gext_ps", tag="gext_ps")
            for cc in range(2):
                nc.tensor.matmul(
                    gext_ps,
                    lhsT=midT_sb[:, cc * 3 : (cc + 1) * 3],
                    rhs=rhs_g[cc],
                    start=(cc == 0),
                    stop=(cc == 1),
                )
            gext_sb = alg.tile([3, C + 1], dt.float32, name="gext_sb")
            nc.vector.tensor_copy(gext_sb, gext_ps)

            # A_ext = mid^T @ g_ext -> [C, 257]; col 256 is v = mid^T u
            A_sb = []
            for cc in range(2):
                A_ps = alg_ps.tile([128, C + 1], dt.float32, name="A_ps", tag="A_ps")
                nc.tensor.matmul(
                    A_ps, lhsT=mid_sb[:, cc * 128 : (cc + 1) * 128], rhs=gext_sb,
                    start=True, stop=True,
                )
                t = alg.tile([128, C + 1], dt.float32, name=f"A{cc}", tag=f"A{cc}")
                nc.vector.tensor_copy(t, A_ps)
                A_sb.append(t)

            # W2T[j][128, 256] via identity-matmul transpose of W2 blocks
            W2T_sb = []
            for jc in range(2):
                W2T_ps = alg_ps.tile([128, C], dt.float32, name="W2T_ps", tag="W2T_ps")
                for oc in range(2):
                    nc.tensor.matmul(
                        W2T_ps[:, oc * 128 : (oc + 1) * 128],
                        lhsT=wo[oc][:, C + jc * 128 : C + (jc + 1) * 128],
                        rhs=ident,
                        start=(oc == 0),
                        stop=(oc == 1),
                    )
                t = alg.tile([128, C], dt.float32, name=f"W2T{jc}", tag=f"W2T{jc}")
                nc.vector.tensor_copy(t, W2T_ps)
                W2T_sb.append(t)

            # WeffT = W1^T + A^T @ W2T  (W1^T added via identity matmuls)
            WeffT_sb = []
            for cc in range(2):
                Wt_ps = alg_ps.tile([128, C], dt.float32, name="Wt_ps", tag="Wt_ps")
                for j in range(2):
                    nc.tensor.matmul(
                        Wt_ps,
                        lhsT=A_sb[j][:, cc * 128 : (cc + 1) * 128],
                        rhs=W2T_sb[j],
                        start=(j == 0),
                        stop=False,
                    )
                for oc in range(2):
                    nc.tensor.matmul(
                        Wt_ps[:, oc * 128 : (oc + 1) * 128],
                        lhsT=wo[oc][:, cc * 128 : (cc + 1) * 128],
                        rhs=ident,
                        start=False,
                        stop=(oc == 1),
                    )
                t = alg.tile([128, C], F32R, name=f"WeffT{cc}", tag=f"WeffT{cc}")
                nc.vector.tensor_copy(t, Wt_ps)
                WeffT_sb.append(t)

            # beff = so * (W2 @ v) + bo
            beff = alg.tile([128, 2], dt.float32, name="beff")
            for oc in range(2):
                wv_ps = alg_ps.tile([128, 1], dt.float32, name="wv_ps", tag="wv_ps")
                for j in range(2):
                    nc.tensor.matmul(
                        wv_ps,
                        lhsT=W2T_sb[j][:, oc * 128 : (oc + 1) * 128],
                        rhs=A_sb[j][:, C : C + 1],
                        start=(j == 0),
                        stop=(j == 1),
                    )
                nc.vector.tensor_scalar(
                    out=beff[:, oc : oc + 1], in0=wv_ps,
                    scalar1=so[:, oc : oc + 1], scalar2=bo[:, oc : oc + 1],
                    op0=OP.mult, op1=OP.add,
                )

        # ---- final: out = so * (Weff @ F) + beff, streamed over n ----
        NT = 512
        with tc.tile_pool(name="fin_ps", bufs=4, space="PSUM") as fin_ps, \
             tc.tile_pool(name="osb", bufs=4) as osb_pool:
            for oc in range(2):
                for nt in range(HW // NT):
                    ps = fin_ps.tile([128, NT], dt.float32, name="ps")
                    for cc in range(2):
                        nc.tensor.matmul(
                            ps,
                            lhsT=WeffT_sb[cc][:, oc * 128 : (oc + 1) * 128],
                            rhs=F[cc][:, nt * NT : (nt + 1) * NT],
                            start=(cc == 0),
                            stop=(cc == 1),
                        )
                    ot = osb_pool.tile([128, NT], dt.float32, name="ot")
                    if nt % 2 == 0:
                        nc.vector.tensor_scalar(
                            out=ot, in0=ps, scalar1=so[:, oc : oc + 1],
                            scalar2=beff[:, oc : oc + 1], op0=OP.mult, op1=OP.add,
                        )
                    else:
                        nc.scalar.activation(
                            out=ot, in_=ps, func=AF.Identity,
                            bias=beff[:, oc : oc + 1], scale=so[:, oc : oc + 1],
                        )
                    nc.sync.dma_start(
                        out=out_d[oc * 128 : (oc + 1) * 128, nt * NT : (nt + 1) * NT],
                        in_=ot,
                    )

    _split_drain_waits(nc)
    return nc


_NC_CACHE = None


def _get_nc():
    global _NC_CACHE
    if _NC_CACHE is None:
        _NC_CACHE = build_nc()
    return _NC_CACHE


def kernel(**inputs):
    feature = np.asarray(inputs["feature"], dtype=np.float32)
    m = np.asarray(inputs["m"], dtype=np.float32)
    shared = {}
    shared["w_feat"] = np.asarray(inputs["w_feat"], dtype=np.float32)
    shared["w_out"] = np.asarray(inputs["w_out"], dtype=np.float32)
    for pre in ("f", "o"):
        for nm in ("gamma", "beta", "mean", "var"):
            key = f"bn_{pre}_{nm}"
            shared[key] = np.asarray(inputs[key], dtype=np.float32)

    nc = _get_nc()
    in_maps = []
    for i in range(NCORES):
        im = dict(shared)
        im["feature"] = np.ascontiguousarray(feature[i].reshape(C, HW))
        im["m"] = np.ascontiguousarray(m[i].reshape(H, W))
        in_maps.append(im)

    res = run_bass_kernel_spmd(nc, in_maps, core_ids=list(range(NCORES)))
    out = np.stack([res.results[i]["out"].reshape(C, H, W) for i in range(NCORES)])
    return out


# revision 12
# speedup vs baseline: 1.0061x; 1.0061x over previous
"""Trainium2 Bass kernel for nn_BDFM_46428596469849.

Per-batch math (B=8, C=256, H=W=128, HW=16384):
    m   = relu(m); z = (m > 0.3)
    er  = minpool4x4(z, SAME, border=1); di = maxpool4x4(z, SAME, border=0)
    fbu = [er, 1-di, di-er]                          # [3, HW]
    mid = fbu @ F^T                                  # [3, C]
    cf  = bn_f(Wf @ F);  mid1 = mid @ cf;  mid2 = mid^T @ mid1
    out = bn_o(W_out @ [F; mid2])

The chain collapses algebraically: with sf/bf (resp. so/bo) the BN scale/bias,
    g    = mid @ (diag(sf) Wf)            # [3, C]
    u    = mid @ bf                       # [3]
    A    = mid^T @ g                      # [C, C]
    v    = mid^T @ u                      # [C]
    Weff = W1 + W2 @ A                    # [C, C]   (W_out = [W1 | W2])
    out  = diag(so) @ Weff @ F + (so*(W2@v) + bo) 1^T
so each batch element needs only: the mid reduction (one pass over F with PE
transposes), tiny C x C algebra, and one C x C x HW matmul streamed over F.

Sharding: data-parallel, one batch element per NeuronCore (8 cores).
"""

import os
import sys

for _p in ("/opt/trn_rl_repo", "/root/.axon_site/_ro/trn_rl_repo"):
    if os.path.isdir(_p) and _p not in sys.path:
        sys.path.insert(0, _p)

import numpy as np

import concourse.bass as bass
import concourse.mybir as mybir
import concourse.tile as tile
from concourse.bass_utils import run_bass_kernel_spmd
from concourse.masks import make_identity

dt = mybir.dt
AF = mybir.ActivationFunctionType
OP = mybir.AluOpType

B, C, H, W = 8, 256, 128, 128
HW = H * W
NCORES = 8
EPS = 1e-5
F32R = dt.float32r


def _split_drain_waits(nc, max_waits=1):
    # Walrus codegen rejects instructions carrying more than a couple of
    # semaphore waits (CTRL drains and DMA descriptors in particular). Hoist
    # excess waits onto preceding NoOps on the same engine queue — the queue
    # executes in order, so the waits are satisfied before the instruction.
    for f in nc.m.functions:
        for bb in f.blocks:
            new_insts = []
            for inst in bb.instructions:
                si = inst.sync_info
                if si is not None and si.on_wait and len(si.on_wait) > max_waits:
                    waits = list(si.on_wait)
                    while len(waits) > max_waits:
                        chunk, waits = waits[:max_waits], waits[max_waits:]
                        pre = mybir.InstNoOp(
                            name=f"I-wsplit-{nc.next_id()}",
                            engine=inst.engine,
                            sync_info=mybir.SyncInfo(on_wait=chunk, on_update=[]),
                        )
                        nc.inst_map[pre.name] = pre
                        new_insts.append(pre)
                    inst.sync_info = mybir.SyncInfo(
                        on_wait=waits, on_update=list(si.on_update)
                    )
                new_insts.append(inst)
            bb.instructions[:] = new_insts


def build_nc():
    from contextlib import ExitStack

    nc = bass.Bass("TRN2", target_bir_lowering=False)

    feat = nc.declare_dram_parameter("feature", [C, HW], dt.float32, isOutput=False)
    m_in = nc.declare_dram_parameter("m", [H, W], dt.float32, isOutput=False)
    wfeat = nc.declare_dram_parameter("w_feat", [C, C], dt.float32, isOutput=False)
    wout = nc.declare_dram_parameter("w_out", [C, 2 * C], dt.float32, isOutput=False)
    bnp = {}
    for pre in ("f", "o"):
        for nm in ("gamma", "beta", "mean", "var"):
            key = f"bn_{pre}_{nm}"
            bnp[key] = nc.declare_dram_parameter(key, [C], dt.float32, isOutput=False)
    out_d = nc.declare_dram_parameter("out", [C, HW], dt.float32, isOutput=True)

    with tile.TileContext(nc) as tc, ExitStack() as ctx:
        const = ctx.enter_context(tc.tile_pool(name="const", bufs=1))
        ident = const.tile([128, 128], dt.float32, name="ident")
        make_identity(nc, ident)
        ident_r = const.tile([128, 128], F32R, name="ident_r")
        nc.vector.tensor_copy(ident_r, ident)
        eps_t = const.tile([128, 1], dt.float32, name="eps_t")
        nc.vector.memset(eps_t, EPS)

        # ---- resident feature in float32r, as per-piece tiles ----
        # DMA brings raw fp32 pieces into a rotating staging pool; an engine
        # copy rounds them into resident f32r tiles (walrus requires f32r
        # matmul operands to be produced pre-rounded by an engine). Separate
        # per-piece tiles keep the dependency granularity fine so the mid
        # phase overlaps the load.
        NPIECE = 16
        PIECE = HW // NPIECE
        fpool = ctx.enter_context(tc.tile_pool(name="fpool", bufs=1))
        F_t = [
            [
                fpool.tile([128, PIECE], F32R, name=f"F{cc}_{i}", tag=f"F{cc}_{i}")
                for i in range(NPIECE)
            ]
            for cc in range(2)
        ]

        def f_slice(cc, col0, width):
            i = col0 // PIECE
            off = col0 % PIECE
            assert off + width <= PIECE
            return F_t[cc][i][:, off : off + width]

        with tc.tile_pool(name="fraw_pool", bufs=6) as fraw_pool:
            for i in range(NPIECE):
                for cc in range(2):
                    fr = fraw_pool.tile([128, PIECE], dt.float32, name="fr")
                    nc.sync.dma_start(
                        out=fr,
                        in_=feat[cc * 128 : (cc + 1) * 128, i * PIECE : (i + 1) * PIECE],
                    )
                    if cc == 0:
                        nc.vector.tensor_copy(F_t[cc][i][:], fr)
                    else:
                        nc.scalar.copy(F_t[cc][i][:], fr)

        # ---- small inputs ----
        wf = []
        wo = []
        for oc in range(2):
            t = const.tile([128, C], dt.float32, name=f"wf{oc}", tag=f"wf{oc}")
            nc.sync.dma_start(out=t, in_=wfeat[oc * 128 : (oc + 1) * 128, :])
            wf.append(t)
            t2 = const.tile([128, 2 * C], dt.float32, name=f"wo{oc}", tag=f"wo{oc}")
            nc.sync.dma_start(out=t2, in_=wout[oc * 128 : (oc + 1) * 128, :])
            wo.append(t2)
        m_sb = const.tile([128, 128], dt.float32, name="m_sb")
        nc.sync.dma_start(out=m_sb, in_=m_in[:, :])

        bnt = {}
        for key, hdl in bnp.items():
            t = const.tile([128, 2], dt.float32, name=f"t_{key}", tag=f"t_{key}")
            nc.sync.dma_start(out=t, in_=hdl[:].rearrange("(t p) -> p t", p=128))
            bnt[key] = t

        # ---- BN scale/bias: s = gamma*rsqrt(var+eps), b = beta - mean*s ----
        setup = ctx.enter_context(tc.tile_pool(name="setup", bufs=1))

        def bn_prep(pre):
            s = setup.tile([128, 2], dt.float32, name=f"s_{pre}", tag=f"s_{pre}")
            b = setup.tile([128, 2], dt.float32, name=f"b_{pre}", tag=f"b_{pre}")
            tmp = setup.tile([128, 2], dt.float32, name=f"tmp_{pre}", tag=f"tmp_{pre}")
            nc.scalar.activation(
                out=tmp, in_=bnt[f"bn_{pre}_var"], func=AF.Sqrt, bias=eps_t, scale=1.0
            )
            nc.vector.reciprocal(out=tmp, in_=tmp)
            nc.vector.tensor_mul(s, bnt[f"bn_{pre}_gamma"], tmp)
            nc.vector.tensor_mul(tmp, bnt[f"bn_{pre}_mean"], s)
            nc.vector.tensor_sub(b, bnt[f"bn_{pre}_beta"], tmp)
            return s, b

        sf, bf = bn_prep("f")
        so, bo = bn_prep("o")

        # ---- morphology: separable 4x4 window (offsets -1..+2), both passes
        # along the free dim with a PE transpose in between; border = the
        # reduction identity (matches reduce_window SAME + init value) ----
        mor = ctx.enter_context(tc.tile_pool(name="mor", bufs=1))

        def pool1d_free(src, op, border, label):
            padd = mor.tile([128, 131], dt.float32, name=f"pad_{label}", tag=f"pad_{label}")
            nc.vector.memset(padd, border)
            nc.vector.tensor_copy(padd[:, 1:129], src)
            a = mor.tile([128, 130], dt.float32, name=f"a_{label}", tag=f"a_{label}")
            nc.vector.tensor_tensor(a, padd[:, 0:130], padd[:, 1:131], op)
            r = mor.tile([128, 128], dt.float32, name=f"r_{label}", tag=f"r_{label}")
            nc.vector.tensor_tensor(r, a[:, 0:128], a[:, 2:130], op)
            return r

        z = mor.tile([128, 128], dt.float32, name="z")
        nc.vector.tensor_scalar(out=z, in0=m_sb, scalar1=0.3, scalar2=None, op0=OP.is_gt)
        erw = pool1d_free(z, OP.min, 1.0, "er1")  # [h, w] pooled over w
        diw = pool1d_free(z, OP.max, 0.0, "di1")
        with tc.tile_pool(name="mor_ps", bufs=1, space="PSUM") as mor_ps:
            er_ps = mor_ps.tile([128, 128], dt.float32, name="er_ps", tag="er_ps")
            nc.tensor.transpose(er_ps, erw, ident)
            erwT = mor.tile([128, 128], dt.float32, name="erwT")
            nc.vector.tensor_copy(erwT, er_ps)
            di_ps = mor_ps.tile([128, 128], dt.float32, name="di_ps", tag="di_ps")
            nc.tensor.transpose(di_ps, diw, ident)
            diwT = mor.tile([128, 128], dt.float32, name="diwT")
            nc.vector.tensor_copy(diwT, di_ps)
        erT = pool1d_free(erwT, OP.min, 1.0, "er2")  # [w, h] pooled over h
        diT = pool1d_free(diwT, OP.max, 0.0, "di2")

        fbuT = mor.tile([128, 128, 3], F32R, name="fbuT")  # [w, h, k]
        nc.vector.tensor_copy(fbuT[:, :, 0], erT)
        nc.vector.tensor_scalar(
            out=fbuT[:, :, 1], in0=diT, scalar1=-1.0, scalar2=1.0, op0=OP.mult, op1=OP.add
        )
        nc.vector.tensor_tensor(fbuT[:, :, 2], diT, erT, OP.subtract)

        # ---- mid = fbu @ F^T via per-h PE transposes, accumulated in PSUM ----
        alg = ctx.enter_context(tc.tile_pool(name="alg", bufs=1))
        mid_sb = alg.tile([3, C], dt.float32, name="mid_sb")
        with tc.tile_pool(name="midps", bufs=1, space="PSUM") as midps:
            mid_ps = midps.tile([3, C], dt.float32, name="mid_ps")
            with tc.tile_pool(name="tr_ps", bufs=3, space="PSUM") as tr_ps_pool, \
                 tc.tile_pool(name="f1T_pool", bufs=3) as f1T_pool:
                for hp in range(64):
                    tps = tr_ps_pool.tile([128, 512], F32R, name="tps")
                    for q in range(4):
                        h = 2 * hp + q // 2
                        cc = q % 2
                        nc.tensor.transpose(
                            tps[:, q * 128 : (q + 1) * 128],
                            f_slice(cc, h * 128, 128),
                            ident_r,
                        )
                    f1T = f1T_pool.tile([128, 512], F32R, name="f1T")
                    if hp % 2 == 0:
                        nc.vector.tensor_copy(f1T, tps)
                    else:
                        nc.scalar.copy(f1T, tps)
                    for q2 in range(2):
                        h = 2 * hp + q2
                        nc.tensor.matmul(
                            mid_ps[:, :],
                            lhsT=fbuT[:, h, :],
                            rhs=f1T[:, q2 * 256 : (q2 + 1) * 256],
                            start=(h == 0),
                            stop=(h == 127),
                        )
            nc.vector.tensor_copy(mid_sb, mid_ps)

        # ---- tiny algebra: g_ext, A_ext, W2T, WeffT, beff (plain fp32) ----
        with tc.tile_pool(name="alg_ps", bufs=1, space="PSUM") as alg_ps:
            midT_sb = alg.tile([128, 6], dt.float32, name="midT_sb")
            for cc in range(2):
                mT2 = alg_ps.tile([128, 3], dt.float32, name="mT2", tag="mT2")
                nc.tensor.transpose(
                    mT2, mid_sb[:, cc * 128 : (cc + 1) * 128], ident[0:3, 0:3]
                )
                nc.vector.tensor_copy(midT_sb[:, cc * 3 : (cc + 1) * 3], mT2)

            # rhs = [diag(sf) Wf | bf] per o-chunk; g_ext = mid @ rhs  -> [3, 257]
            rhs_g = []
            for cc in range(2):
                r = alg.tile([128, C + 1], dt.float32, name=f"rhs_g{cc}", tag=f"rhs_g{cc}")
                nc.vector.tensor_scalar(
                    out=r[:, 0:C], in0=wf[cc], scalar1=sf[:, cc : cc + 1],
                    scalar2=None, op0=OP.mult,
                )
                nc.vector.tensor_copy(r[:, C : C + 1], bf[:, cc : cc + 1])
                rhs_g.append(r)
            gext_ps = alg_ps.tile([3, C + 1], dt.float32, name="gext_ps", tag="gext_ps")
            for cc in range(2):
                nc.tensor.matmul(
                    gext_ps,
                    lhsT=midT_sb[:, cc * 3 : (cc + 1) * 3],
                    rhs=rhs_g[cc],
                    start=(cc == 0),
                    stop=(cc == 1),
                )
            gext_sb = alg.tile([3, C + 1], dt.float32, name="gext_sb")
            nc.vector.tensor_copy(gext_sb, gext_ps)

            # A_ext = mid^T @ g_ext -> [C, 257]; col 256 is v = mid^T u
            A_sb = []
            for cc in range(2):
                A_ps = alg_ps.tile([128, C + 1], dt.float32, name="A_ps", tag="A_ps")
                nc.tensor.matmul(
                    A_ps, lhsT=mid_sb[:, cc * 128 : (cc + 1) * 128], rhs=gext_sb,
                    start=True, stop=True,
                )
                t = alg.tile([128, C + 1], dt.float32, name=f"A{cc}", tag=f"A{cc}")
                nc.vector.tensor_copy(t, A_ps)
                A_sb.append(t)

            # W2T[j][128, 256] via identity-matmul transpose of W2 blocks
            W2T_sb = []
            for jc in range(2):
                W2T_ps = alg_ps.tile([128, C], dt.float32, name="W2T_ps", tag="W2T_ps")
                for oc in range(2):
                    nc.tensor.matmul(
                        W2T_ps[:, oc * 128 : (oc + 1) * 128],
                        lhsT=wo[oc][:, C + jc * 128 : C + (jc + 1) * 128],
                        rhs=ident,
                        start=(oc == 0),
                        stop=(oc == 1),
                    )
                t = alg.tile([128, C], dt.float32, name=f"W2T{jc}", tag=f"W2T{jc}")
                nc.vector.tensor_copy(t, W2T_ps)
                W2T_sb.append(t)

            # WeffT = W1^T + A^T @ W2T  (W1^T added via identity matmuls)
            WeffT_sb = []
            for cc in range(2):
                Wt_ps = alg_ps.tile([128, C], dt.float32, name="Wt_ps", tag="Wt_ps")
                for j in range(2):
                    nc.tensor.matmul(
                        Wt_ps,
                        lhsT=A_sb[j][:, cc * 128 : (cc + 1) * 128],
                        rhs=W2T_sb[j],
                        start=(j == 0),
                        stop=False,
                    )
                for oc in range(2):
                    nc.tensor.matmul(
                        Wt_ps[:, oc * 128 : (oc + 1) * 128],
                        lhsT=wo[oc][:, cc * 128 : (cc + 1) * 128],
                        rhs=ident,
                        start=False,
                        stop=(oc == 1),
                    )
                t = alg.tile([128, C], F32R, name=f"WeffT{cc}", tag=f"WeffT{cc}")
                nc.vector.tensor_copy(t, Wt_ps)
                WeffT_sb.append(t)

            # beff = so * (W2 @ v) + bo
            beff = alg.tile([128, 2], dt.float32, name="beff")
            for oc in range(2):
                wv_ps = alg_ps.tile([128, 1], dt.float32, name="wv_ps", tag="wv_ps")
                for j in range(2):
                    nc.tensor.matmul(
                        wv_ps,
                        lhsT=W2T_sb[j][:, oc * 128 : (oc + 1) * 128],
                        rhs=A_sb[j][:, C : C + 1],
                        start=(j == 0),
                        stop=(j == 1),
                    )
                nc.vector.tensor_scalar(
                    out=beff[:, oc : oc + 1], in0=wv_ps,
                    scalar1=so[:, oc : oc + 1], scalar2=bo[:, oc : oc + 1],
                    op0=OP.mult, op1=OP.add,
                )

        # ---- final: out = so * (Weff @ F) + beff, streamed over n ----
        # Groups of 4 n-tiles with cc outer keep one stationary operand for
        # runs of 4 matmuls; 8 PSUM banks double-buffer across groups.
        NT = 512
        GRP = 4
        with tc.tile_pool(name="fin_ps", bufs=8, space="PSUM") as fin_ps, \
             tc.tile_pool(name="osb", bufs=6) as osb_pool:
            for oc in range(2):
                for g in range(HW // NT // GRP):
                    pss = [
                        fin_ps.tile([128, NT], dt.float32, name="ps", tag="ps")
                        for _ in range(GRP)
                    ]
                    for cc in range(2):
                        for t in range(GRP):
                            nt = g * GRP + t
                            nc.tensor.matmul(
                                pss[t],
                                lhsT=WeffT_sb[cc][:, oc * 128 : (oc + 1) * 128],
                                rhs=f_slice(cc, nt * NT, NT),
                                start=(cc == 0),
                                stop=(cc == 1),
                            )
                    for t in range(GRP):
                        nt = g * GRP + t
                        ot = osb_pool.tile([128, NT], dt.float32, name="ot")
                        if t % 2 == 0:
                            nc.vector.tensor_scalar(
                                out=ot, in0=pss[t], scalar1=so[:, oc : oc + 1],
                                scalar2=beff[:, oc : oc + 1], op0=OP.mult, op1=OP.add,
                            )
                        else:
                            nc.scalar.activation(
                                out=ot, in_=pss[t], func=AF.Identity,
                                bias=beff[:, oc : oc + 1], scale=so[:, oc : oc + 1],
                            )
                        nc.sync.dma_start(
                            out=out_d[oc * 128 : (oc + 1) * 128, nt * NT : (nt + 1) * NT],
                            in_=ot,
                        )

    _split_drain_waits(nc)
    return nc


_NC_CACHE = None


def _get_nc():
    global _NC_CACHE
    if _NC_CACHE is None:
        _NC_CACHE = build_nc()
    return _NC_CACHE


def kernel(**inputs):
    feature = np.asarray(inputs["feature"], dtype=np.float32)
    m = np.asarray(inputs["m"], dtype=np.float32)
    shared = {}
    shared["w_feat"] = np.asarray(inputs["w_feat"], dtype=np.float32)
    shared["w_out"] = np.asarray(inputs["w_out"], dtype=np.float32)
    for pre in ("f", "o"):
        for nm in ("gamma", "beta", "mean", "var"):
            key = f"bn_{pre}_{nm}"
            shared[key] = np.asarray(inputs[key], dtype=np.float32)

    nc = _get_nc()
    in_maps = []
    for i in range(NCORES):
        im = dict(shared)
        im["feature"] = np.ascontiguousarray(feature[i].reshape(C, HW))
        im["m"] = np.ascontiguousarray(m[i].reshape(H, W))
        in_maps.append(im)

    res = run_bass_kernel_spmd(nc, in_maps, core_ids=list(range(NCORES)))
    out = np.stack([res.results[i]["out"].reshape(C, H, W) for i in range(NCORES)])
    return out
